# revision 14
# baseline (speedup 1.0000x reference)
"""Trainium2 Bass kernel for the 20-layer dilated-causal-conv audio model.

Formulation (validated against the reference in numpy):
- Only the last 128 output timesteps are needed -> per-layer suffix pyramid.
  Layer i only computes timesteps in blocks [TB[i+1], 512) of 16 steps each.
- Channels are tiny (8), so convs run on the TensorEngine as block-Toeplitz
  matmuls: partition dim = 16 timesteps x 8 channels = 128; each conv tap is a
  host-built 128x128 stationary matrix; taps accumulate in PSUM.
- The per-layer control (1x1 conv on ctrl) is one extra matmul with
  contraction 16; all per-channel constants (conv bias, ctrl bias, folded
  io_b drift) ride the ReLU activation's per-partition bias.
- Residual 1x1 (io_w) is a block-diagonal matmul; the residual add runs on
  the VectorEngine. The final mixer is a per-layer [128,16] matmul over the
  last 8 blocks, accumulated on the VectorEngine.
- Data parallel over batch: 32 batches -> 8 cores x 4.
"""

import numpy as np

import concourse.bass as bass
import concourse.mybir as mybir
import concourse.tile as tile
from concourse.bass_utils import run_bass_kernel_spmd

# ---------------------------------------------------------------- constants
DIL = [1, 2, 4, 8, 16, 32, 64, 128, 256, 512] * 2
NL = 20          # layers
CH = 8           # channels
BLK = 16         # timesteps per block
NB = 512         # blocks in T=8192
T = 8192
B = 32           # total batch
NCORES = 8
BPC = B // NCORES  # batch per core
CHUNK = 128      # output blocks per chunk (=> matmul free dim 4*128 = 512)

# matmul dtype: float32r = single-pass reduced-precision fp32 (fp32 storage)
DT = mybir.dt.float32

# block-start table: TB[i] = first block of x~_i ; TB[NL] = first output block
TB = [0] * (NL + 1)
TB[NL] = NB - 8
for i in range(NL - 1, -1, -1):
    TB[i] = TB[i + 1] - max(1, (2 * DIL[i]) // BLK)

# per-layer tap metadata: list of (block_offset, tap_array_index)
_TAP_OFFSETS = []
_NT = 0
for _i in range(NL):
    d = DIL[_i]
    offs = [0, d // BLK, 2 * (d // BLK)] if d >= BLK else [0, 1]
    _TAP_OFFSETS.append([(o, _NT + j) for j, o in enumerate(offs)])
    _NT += len(offs)


# ------------------------------------------------- workaround: 1-wait limit
def _split_multi_waits(nc):
    """This walrus build allows only one sem wait per TPB instruction, but
    Tile's kernel-tail drain carries several. Move extras onto preceding
    same-engine nops (in-order execution keeps the gating semantics)."""
    tpb = {
        mybir.EngineType.SP,
        mybir.EngineType.PE,
        mybir.EngineType.DVE,
        mybir.EngineType.Activation,
        mybir.EngineType.Pool,
    }
    for f in nc.m.functions:
        for bb in f.blocks:
            new_list = []
            changed = False
            for inst in bb.instructions:
                si = inst.sync_info
                if si is not None and si.on_wait and len(si.on_wait) > 1 and inst.engine in tpb:
                    waits = list(si.on_wait)
                    for j, w in enumerate(waits[:-1]):
                        nop = mybir.InstNoOp(name=f"{inst.name}-ws{j}", ins=[], outs=[])
                        nop.engine = inst.engine
                        nop.sync_info = mybir.SyncInfo(on_wait=[w], on_update=[])
                        new_list.append(nop)
                    si.on_wait = waits[-1:]
                    changed = True
                new_list.append(inst)
            if changed:
                bb.instructions[:] = new_list


# ------------------------------------------------------------- host arrays
def _build_host_arrays(inputs):
    c_w0 = np.asarray(inputs["c_w0"], np.float32)    # [3,1,8]
    c_ws = np.asarray(inputs["c_ws"], np.float32)    # [19,3,8,8]
    c_b = np.asarray(inputs["c_b"], np.float32)      # [20,8]
    ctrl_w = np.asarray(inputs["ctrl_w"], np.float32)  # [20,1,1]
    ctrl_b = np.asarray(inputs["ctrl_b"], np.float32)  # [20,1]
    io_w = np.asarray(inputs["io_w"], np.float32)    # [19,8,8]
    io_b = np.asarray(inputs["io_b"], np.float32)    # [19,8]
    mix_w = np.asarray(inputs["mix_w"], np.float32)  # [160,1]

    tapw = np.zeros((_NT, 128, 128), np.float32)
    iow = np.zeros((NL - 1, 128, 128), np.float32)
    # auxw[NL] is the audio channel-broadcast matrix for the layer-0 residual
    auxw = np.zeros((NL + 1, 16, 128), np.float32)
    mixw = np.zeros((NL, 128, 16), np.float32)
    biases = np.zeros((128, NL), np.float32)
    for t in range(BLK):
        auxw[NL, t, t * 8 : t * 8 + 8] = 1.0

    const_i = np.zeros(CH, np.float32)
    for i in range(NL):
        w = c_w0 if i == 0 else c_ws[i - 1]          # [3, cin, 8]
        cin = w.shape[1]
        d = DIL[i]
        wD = [w[2], w[1], w[0]]                      # wD[l] multiplies x[t - l*d]
        bias = c_b[i] + ctrl_b[i][0]
        if cin == CH:
            bias = bias + np.einsum("kco,c->o", w, const_i)
        biases[:, i] = np.tile(bias, BLK)

        # layer 0 (cin=1) reads the 16-partition audio tile: row index = ti.
        # layers >0 read the 128-partition x~ tile: row index = ti*8 + ci.
        def rows(ti):
            return slice(ti, ti + 1) if cin == 1 else slice(ti * 8, ti * 8 + cin)

        if d >= BLK:
            for l, (_, idx) in enumerate(_TAP_OFFSETS[i]):
                W = tapw[idx]
                for t in range(BLK):
                    W[rows(t), t * 8 : t * 8 + 8] = wD[l][:cin]
        else:
            Wc = tapw[_TAP_OFFSETS[i][0][1]]
            Wp = tapw[_TAP_OFFSETS[i][1][1]]
            for to in range(BLK):
                for l in range(3):
                    ti = to - l * d
                    if ti >= 0:
                        Wc[rows(ti), to * 8 : to * 8 + 8] += wD[l][:cin]
                    else:
                        Wp[rows(ti + BLK), to * 8 : to * 8 + 8] += wD[l][:cin]

        for t in range(BLK):
            auxw[i, t, t * 8 : t * 8 + 8] = ctrl_w[i][0, 0]
            mixw[i, t * 8 : t * 8 + 8, t] = mix_w[i * 8 : i * 8 + 8, 0]
        if i < NL - 1:
            for t in range(BLK):
                iow[i, t * 8 : t * 8 + 8, t * 8 : t * 8 + 8] = io_w[i]
            const_i = const_i + io_b[i]

    return dict(tapw=tapw, iow=iow, auxw=auxw, mixw=mixw, biases=biases)


# ----------------------------------------------------------- device program
_NC_CACHE = {}


def _chunks(out_b):
    """Right-aligned chunks over output blocks [out_b, NB): list of (lo, w),
    left to right; the rightmost chunk always covers the final 8 blocks."""
    out = []
    hi = NB
    while hi > out_b:
        lo = max(out_b, hi - CHUNK)
        out.append((lo, hi - lo))
        hi = lo
    return out[::-1]


def _build_nc():
    nc = bass.Bass()
    f32 = mybir.dt.float32

    nblk0 = NB - TB[0]
    nblk1 = NB - TB[1]
    # audio/ctrl arrive host-blocked as [16=t-in-block, BPC, nblk]
    audio_h = nc.dram_tensor("audio", [BLK, BPC, nblk0], DT, kind="ExternalInput")
    ctrl_h = nc.dram_tensor("ctrl", [BLK, BPC, nblk1], DT, kind="ExternalInput")
    tapw_h = nc.dram_tensor("tapw", [_NT, 128, 128], DT, kind="ExternalInput")
    iow_h = nc.dram_tensor("iow", [NL - 1, 128, 128], DT, kind="ExternalInput")
    auxw_h = nc.dram_tensor("auxw", [NL + 1, 16, 128], DT, kind="ExternalInput")
    mixw_h = nc.dram_tensor("mixw", [NL, 128, 16], DT, kind="ExternalInput")
    biases_h = nc.dram_tensor("biases", [128, NL], DT, kind="ExternalInput")
    out_h = nc.dram_tensor("out", [BPC, 128], f32, kind="ExternalOutput")

    with tile.TileContext(nc) as tc:
        with (
            tc.tile_pool(name="w", bufs=1) as wpool,
            tc.tile_pool(name="xs", bufs=1) as xpool,
            tc.tile_pool(name="h", bufs=4) as hpool,
            tc.tile_pool(name="pc", bufs=3, space="PSUM") as pcpool,
            tc.tile_pool(name="pio", bufs=3, space="PSUM") as piopool,
            tc.tile_pool(name="pm", bufs=2, space="PSUM") as pmpool,
        ):
            # ---- weights & inputs to SBUF
            tapw_t = wpool.tile([128, _NT, 128], DT)
            nc.sync.dma_start(out=tapw_t[:], in_=tapw_h[:].rearrange("m p c -> p m c"))
            iow_t = wpool.tile([128, NL - 1, 128], DT)
            nc.sync.dma_start(out=iow_t[:], in_=iow_h[:].rearrange("m p c -> p m c"))
            auxw_t = wpool.tile([16, NL + 1, 128], DT)
            nc.sync.dma_start(out=auxw_t[:], in_=auxw_h[:].rearrange("m p c -> p m c"))
            mixw_t = wpool.tile([128, NL, 16], DT)
            nc.sync.dma_start(out=mixw_t[:], in_=mixw_h[:].rearrange("m p c -> p m c"))
            bias_t = wpool.tile([128, NL], DT)
            nc.sync.dma_start(out=bias_t[:], in_=biases_h[:])

            # audio blocks: [16=t, BPC, nblk0] (layer 0 has cin=1; the
            # channel broadcast happens in layer 0's residual matmul)
            audio_t = xpool.tile([16, BPC, nblk0], DT, tag="x0")
            nc.sync.dma_start(out=audio_t[:], in_=audio_h[:])

            # ctrl blocks: [16=t, BPC, nblk1]
            ctrl_t = wpool.tile([16, BPC, nblk1], DT)
            nc.sync.dma_start(out=ctrl_t[:], in_=ctrl_h[:])

            # mix accumulator
            macc = wpool.tile([16, BPC, 8], f32)
            nc.vector.memset(macc, 0.0)

            x_t = audio_t
            for i in range(NL):
                out_b = TB[i + 1]
                x_next = None
                if i < NL - 1:
                    x_next = xpool.tile(
                        [128, BPC, NB - out_b], DT, tag=f"x{i + 1}", name=f"x{i + 1}"
                    )
                kp = 16 if i == 0 else 128  # contraction rows of x~_i
                for lo, w in _chunks(out_b):
                    pc = pcpool.tile([128, BPC, CHUNK], f32)
                    for j, (off, idx) in enumerate(_TAP_OFFSETS[i]):
                        a = lo - off - TB[i]
                        nc.tensor.matmul(
                            pc[:, :, :w],
                            tapw_t[:kp, idx, :],
                            x_t[:, :, a : a + w],
                            start=(j == 0),
                            stop=False,
                        )
                    a = lo - TB[1]
                    nc.tensor.matmul(
                        pc[:, :, :w],
                        auxw_t[:, i, :],
                        ctrl_t[:, :, a : a + w],
                        start=False,
                        stop=True,
                    )
                    h = hpool.tile([128, BPC, CHUNK], DT)
                    nc.scalar.activation(
                        out=h[:, :, :w],
                        in_=pc[:, :, :w],
                        func=mybir.ActivationFunctionType.Relu,
                        bias=bias_t[:, i : i + 1],
                        scale=1.0,
                    )
                    if lo + w == NB:  # rightmost chunk: mixer contribution
                        pm = pmpool.tile([16, BPC, 8], f32)
                        nc.tensor.matmul(
                            pm[:],
                            mixw_t[:, i, :],
                            h[:, :, w - 8 : w],
                            start=True,
                            stop=True,
                        )
                        nc.vector.tensor_add(out=macc[:], in0=macc[:], in1=pm[:])
                    if i < NL - 1:
                        pio = piopool.tile([128, BPC, CHUNK], f32)
                        if i == 0:
                            # x~_1 = h0 @ io_0 + audio (channel-broadcast via
                            # an extra contraction-16 matmul), copied out
                            nc.tensor.matmul(
                                pio[:, :, :w],
                                iow_t[:, i, :],
                                h[:, :, :w],
                                start=True,
                                stop=False,
                            )
                            a = lo - TB[0]
                            nc.tensor.matmul(
                                pio[:, :, :w],
                                auxw_t[:, NL, :],
                                audio_t[:, :, a : a + w],
                                start=False,
                                stop=True,
                            )
                            nc.vector.tensor_copy(
                                out=x_next[:, :, lo - out_b : lo - out_b + w],
                                in_=pio[:, :, :w],
                            )
                        else:
                            nc.tensor.matmul(
                                pio[:, :, :w],
                                iow_t[:, i, :],
                                h[:, :, :w],
                                start=True,
                                stop=True,
                            )
                            nc.vector.tensor_add(
                                out=x_next[:, :, lo - out_b : lo - out_b + w],
                                in0=x_t[:, :, lo - TB[i] : lo - TB[i] + w],
                                in1=pio[:, :, :w],
                            )
                x_t = x_next

            # out: [16, BPC, 8] -> DRAM [BPC, 128]
            out_t = wpool.tile([16, BPC, 8], f32)
            nc.scalar.activation(
                out=out_t[:],
                in_=macc[:],
                func=mybir.ActivationFunctionType.Copy,
            )
            dst = bass.AP(tensor=out_h, offset=0, ap=[[1, BLK], [128, BPC], [BLK, 8]])
            nc.sync.dma_start(out=dst, in_=out_t[:])

    _split_multi_waits(nc)
    return nc


def _get_nc():
    if "nc" not in _NC_CACHE:
        _NC_CACHE["nc"] = _build_nc()
    return _NC_CACHE["nc"]


# ------------------------------------------------------------------- public
def _block(sig, b0):
    """[b, T] -> [16, b, nblk] suffix-block layout starting at block b0."""
    nblk = NB - b0
    v = sig[:, b0 * BLK :].reshape(sig.shape[0], nblk, BLK)
    return np.ascontiguousarray(v.transpose(2, 0, 1))


def kernel(**inputs) -> np.ndarray:
    nc = _get_nc()
    host = _build_host_arrays(inputs)
    audio = np.asarray(inputs["audio"], np.float32)[:, :, 0]
    ctrl = np.asarray(inputs["ctrl"], np.float32)[:, :, 0]
    mix_b = float(np.asarray(inputs["mix_b"], np.float32)[0])

    in_maps = []
    for c in range(NCORES):
        sl = slice(c * BPC, (c + 1) * BPC)
        in_maps.append(
            {
                "audio": _block(audio[sl], TB[0]),
                "ctrl": _block(ctrl[sl], TB[1]),
                "tapw": host["tapw"],
                "iow": host["iow"],
                "auxw": host["auxw"],
                "mixw": host["mixw"],
                "biases": host["biases"],
            }
        )
    res = run_bass_kernel_spmd(nc, in_maps, core_ids=list(range(NCORES)))
    out = np.concatenate([res.results[c]["out"] for c in range(NCORES)], axis=0)
    return (out + mix_b).astype(np.float32)
